# revision 2
# baseline (speedup 1.0000x reference)
"""Trainium2 Bass kernel: 3x3 VALID conv (NHWC, 256->256 ch) with weight
thresholding + bias, batch-sharded across 8 NeuronCores (4 images/core).

Device strategy per core:
  - x pre-transposed on host to [cin, H*W] (2 partition tiles of 128),
    loaded per image in 4 row-aligned chunks (16 out-rows each) so compute
    starts early and chunks double-buffer.
  - conv = 9 shifted matmuls per output tile accumulated in PSUM over
    9 taps x 2 cin tiles, fp32r (1 cyc/row, TF32-class precision).
  - moving operand is a 3D AP [128, rows, 62] with row stride 64: only the
    62 valid output columns per row are computed (packed output, no
    garbage columns, no padding needed).
  - bias fused into the PSUM->SBUF drain (DVE tensor_scalar_add).
"""

import sys

sys.path.insert(0, "/opt/trn_rl_repo")

import numpy as np

import concourse.bacc as bacc
import concourse.mybir as mybir
import concourse.tile as tile
from concourse.bass_utils import run_bass_kernel_spmd

F32 = mybir.dt.float32
F32R = mybir.dt.float32r

N_CORES = 8
IMG_PER_CORE = 4
C = 256
NPIX = 4096               # 64*64 input pixels per image
NV = 62 * 62              # 3844 valid output pixels per image
# 4 input-row chunks per image: (first_input_row, n_input_rows)
CHUNKS = [(0, 18), (16, 18), (32, 18), (48, 16)]
# output blocks: (out_row0, n_out_rows, chunk_idx)
BLOCKS = [(8 * b, 8 if b < 7 else 6, b // 2) for b in range(8)]
SPARSE_TH = 0.01
TAPS = [(kh, kw) for kh in range(3) for kw in range(3)]

_CACHE = {}


def _emit_rep(nc, x_d, o_d, w_sb, b_sb, xp, pp, op):
    """One full pass: conv of IMG_PER_CORE images (the per-rep body)."""
    for img in range(IMG_PER_CORE):
        x_sb = [[None] * 4 for _ in range(2)]
        for ci, (r0, nr) in enumerate(CHUNKS):
            for ct in range(2):
                xt = xp.tile([128, nr, 64], F32R,
                             tag=f"x{ct}c{ci}")
                nc.sync.dma_start(
                    xt[:], x_d[img, ct, :, r0 * 64:(r0 + nr) * 64])
                x_sb[ct][ci] = xt
        for y0, nrow, ci in BLOCKS:
            n = 62 * nrow
            p0 = 62 * y0
            lr = y0 - CHUNKS[ci][0]
            for co in range(2):
                ps = pp.tile([128, n], F32, tag="ps")
                for ct in range(2):
                    for t, (kh, kw) in enumerate(TAPS):
                        nc.tensor.matmul(
                            ps[:],
                            w_sb[ct][:, t * C + co * 128:
                                     t * C + co * 128 + 128],
                            x_sb[ct][ci][:, lr + kh:lr + kh + nrow,
                                         kw:kw + 62],
                            start=(ct == 0 and t == 0),
                            stop=(ct == 1 and t == 8),
                        )
                ob = op.tile([128, n], F32, tag="ob")
                nc.vector.tensor_scalar_add(
                    ob[:], ps[:], b_sb[:, co:co + 1])
                nc.sync.dma_start(o_d[img, co, :, p0:p0 + n],
                                  ob[:])


def _build(reps: int = 1, hw_loop: bool = False):
    key = (reps, hw_loop)
    if key in _CACHE:
        return _CACHE[key]

    nc = bacc.Bacc("TRN2", target_bir_lowering=False, debug=False,
                   num_devices=N_CORES)

    x_d = nc.dram_tensor("xt", [IMG_PER_CORE, 2, 128, NPIX], F32R,
                         kind="ExternalInput")
    w_d = nc.dram_tensor("wt", [2, 128, 9 * C], F32R, kind="ExternalInput")
    b_d = nc.dram_tensor("bias", [128, 2], F32, kind="ExternalInput")
    o_d = nc.dram_tensor("out", [IMG_PER_CORE, 2, 128, NV], F32,
                         kind="ExternalOutput")

    with tile.TileContext(nc) as tc:
        with tc.tile_pool(name="wp", bufs=1) as wp, \
             tc.tile_pool(name="xp", bufs=2) as xp, \
             tc.tile_pool(name="pp", bufs=8, space="PSUM") as pp, \
             tc.tile_pool(name="op", bufs=6) as op:

            w_sb = []
            for ct in range(2):
                wt = wp.tile([128, 9 * C], F32R, tag=f"w{ct}")
                nc.sync.dma_start(wt[:], w_d[ct])
                w_sb.append(wt)
            b_sb = wp.tile([128, 2], F32, tag="bias")
            nc.sync.dma_start(b_sb[:], b_d[:])

            if hw_loop and reps > 1:
                with tc.For_i(0, reps, 1):
                    _emit_rep(nc, x_d, o_d, w_sb, b_sb, xp, pp, op)
            else:
                for _ in range(reps):
                    _emit_rep(nc, x_d, o_d, w_sb, b_sb, xp, pp, op)

    nc.compile()
    _CACHE[key] = nc
    return nc


def _prep_inputs(x, weight, bias):
    """Host-side shard prep: threshold mask + relayout. Per-core in_maps."""
    w = np.where(np.abs(weight) < SPARSE_TH, 0.0, weight).astype(np.float32)
    # (cout, cin, kh, kw) -> (cin, kh, kw, cout) -> [2, 128, 9*256]
    wt = np.ascontiguousarray(w.transpose(1, 2, 3, 0)).reshape(2, 128, 9 * C)
    b2 = np.ascontiguousarray(bias.astype(np.float32).reshape(2, 128).T)

    n_img = x.shape[0]
    xs = np.ascontiguousarray(
        x.astype(np.float32).reshape(n_img, NPIX, C).transpose(0, 2, 1))
    xs = xs.reshape(n_img, 2, 128, NPIX)

    in_maps = []
    for c in range(N_CORES):
        in_maps.append({
            "xt": np.ascontiguousarray(
                xs[c * IMG_PER_CORE:(c + 1) * IMG_PER_CORE]),
            "wt": wt,
            "bias": b2,
        })
    return in_maps


def _assemble(results):
    outs = np.concatenate([r["out"] for r in results], axis=0)  # (32,2,128,3844)
    outs = outs.reshape(32, C, 62, 62).transpose(0, 2, 3, 1)
    return np.ascontiguousarray(outs)


def kernel(x, weight, bias):
    x = np.asarray(x)
    weight = np.asarray(weight)
    bias = np.asarray(bias)
    nc = _build(reps=1)
    in_maps = _prep_inputs(x, weight, bias)
    res = run_bass_kernel_spmd(nc, in_maps, list(range(N_CORES)))
    return _assemble(res.results)



# revision 4
# speedup vs baseline: 1.2818x; 1.2818x over previous
"""Trainium2 Bass kernel: 3x3 VALID conv (NHWC, 256->256 ch) with weight
thresholding + bias, batch-sharded across 8 NeuronCores (4 images/core).

Algorithm: vertical 1D Winograd F(4,3) over kh — halves tensor-engine work
vs direct conv (6 wino positions replace 3 kh taps per 4 output rows,
i.e. 12 -> 6 matmul-rows per output pixel after the co/ct double-stream).

Per-core pipeline (all fp16 operands, fp32 PSUM accumulation):
  - x host-prepped to [img, cin-tile, 128, 66*64] fp16 (2 zero pad rows).
  - DVE input transform: V[p] = BT-combination of 6 strided row-planes,
    12 fused ops per (img, cin-tile), fp16 2x mode.
  - PE: per (chunk of 8 vtiles, co, p): 6 accumulating fp16 matmuls
    (2 cin-tiles x 3 kw shifts), moving operand [128, 8, 62] strided AP,
    496 rows each. Host-side G-transformed weights, FWL fast loads.
  - ACT drains PSUM -> fp16 M tiles (bias pre-added into M0/M5 lanes).
  - DVE inverse transform: out rows = AT-combination, 12 fused ops per
    (chunk, co), fp16 out; host casts to fp32.
"""

import sys

sys.path.insert(0, "/opt/trn_rl_repo")

import numpy as np

import concourse.bacc as bacc
import concourse.mybir as mybir
import concourse.tile as tile
from concourse.bass_utils import run_bass_kernel_spmd

F32 = mybir.dt.float32
F16 = mybir.dt.float16
ADD = mybir.AluOpType.add
SUB = mybir.AluOpType.subtract
MULT = mybir.AluOpType.mult
COPY = mybir.ActivationFunctionType.Copy
IDENT = mybir.ActivationFunctionType.Identity

N_CORES = 8
IMG_PER_CORE = 4
C = 256
NV = 62 * 62              # 3844 valid output pixels per image
NXROW = 66                # 64 input rows + 2 zero pad rows
SPARSE_TH = 0.01

# F(4,3) transform matrices (Lavin), float64 on host
G_MAT = np.array([
    [1 / 4, 0, 0],
    [-1 / 6, -1 / 6, -1 / 6],
    [-1 / 6, 1 / 6, -1 / 6],
    [1 / 24, 1 / 12, 1 / 6],
    [1 / 24, -1 / 12, 1 / 6],
    [0, 0, 1],
], dtype=np.float64)

_CACHE = {}


def _emit_rep(nc, x_d, o_d, w_sb, b_sb, xp, vp, tp, pp, mp, op):
    for img in range(IMG_PER_CORE):
        x_sb = []
        for ct in range(2):
            xt = xp.tile([128, NXROW, 64], F16, tag=f"x{ct}")
            nc.sync.dma_start(xt[:], x_d[img, ct])
            x_sb.append(xt)

        # ---- input transform: V[p] over 16 vtiles of 4 rows ----
        v_sb = []
        for ct in range(2):
            vt = vp.tile([128, 6, 16, 64], F16, tag=f"v{ct}")
            t1 = tp.tile([128, 16, 64], F16, tag=f"t1{ct}")
            t2 = tp.tile([128, 16, 64], F16, tag=f"t2{ct}")

            def d(r, ct=ct):
                return x_sb[ct][:, r:r + 61:4, :]

            V = lambda p, vt=vt: vt[:, p]
            stt = nc.vector.scalar_tensor_tensor
            tt = nc.vector.tensor_tensor
            # V0 = 4 d0 - 5 d2 + d4 ; V5 = 4 d1 - 5 d3 + d5
            stt(t1[:], d(2), -5.0, d(4), MULT, ADD)
            stt(V(0), d(0), 4.0, t1[:], MULT, ADD)
            stt(t2[:], d(3), -5.0, d(5), MULT, ADD)
            stt(V(5), d(1), 4.0, t2[:], MULT, ADD)
            # s = d4 - 4 d2 ; p = 4 d1 - d3 ; V1 = s - p ; V2 = s + p
            stt(t1[:], d(2), -4.0, d(4), MULT, ADD)
            stt(t2[:], d(1), 4.0, d(3), MULT, SUB)
            tt(V(1), t1[:], t2[:], SUB)
            tt(V(2), t1[:], t2[:], ADD)
            # u = d1 - d3 ; v = d4 - d2 ; V3 = -2u + v ; V4 = 2u + v
            tt(t1[:], d(1), d(3), SUB)
            tt(t2[:], d(4), d(2), SUB)
            stt(V(3), t1[:], -2.0, t2[:], MULT, ADD)
            stt(V(4), t1[:], 2.0, t2[:], MULT, ADD)
            v_sb.append(vt)

        # ---- matmuls + drains + inverse transform ----
        for chunk in range(2):
            vt0 = chunk * 8
            for co in range(2):
                m = mp.tile([128, 6, 496], F16, tag="m")
                for p in range(6):
                    ps = pp.tile([128, 496], F32, tag="ps")
                    k = 0
                    for ct in range(2):
                        for kw in range(3):
                            off = ct * 4608 + p * 768 + kw * 256 + co * 128
                            nc.tensor.matmul(
                                ps[:],
                                w_sb[:, off:off + 128],
                                v_sb[ct][:, p, vt0:vt0 + 8, kw:kw + 62],
                                start=(k == 0), stop=(k == 5),
                            )
                            k += 1
                    if p in (0, 5):
                        nc.scalar.activation(m[:, p], ps[:], IDENT,
                                             bias=b_sb[:, co:co + 1])
                    else:
                        nc.scalar.activation(m[:, p], ps[:], COPY)

                # inverse: out[4t+i] = AT[i] . M[:, t]  (+bias in M0/M5)
                ob = op.tile([128, 8, 4, 62], F16, tag="ob")
                i1 = tp.tile([128, 496], F16, tag="i1")
                i2 = tp.tile([128, 496], F16, tag="i2")
                i3 = tp.tile([128, 496], F16, tag="i3")
                M = lambda p: m[:, p]
                O = lambda i: ob[:, :, i, :]
                stt = nc.vector.scalar_tensor_tensor
                tt = nc.vector.tensor_tensor
                # out0 = M0b + (M1+M2) + (M3+M4)
                tt(i1[:], M(1), M(2), ADD)        # t1
                tt(i2[:], M(3), M(4), ADD)        # t2
                tt(i3[:], M(0), i1[:], ADD)
                tt(O(0), i3[:], i2[:], ADD)
                # out2 = 4*t2 + t1 + bias
                stt(i3[:], i2[:], 4.0, i1[:], MULT, ADD)
                nc.vector.tensor_scalar_add(O(2), i3[:], b_sb[:, co:co + 1])
                # u = M1-M2 ; v = M3-M4
                tt(i1[:], M(1), M(2), SUB)        # u
                tt(i2[:], M(3), M(4), SUB)        # v
                # out1 = 2v + u + bias
                stt(i3[:], i2[:], 2.0, i1[:], MULT, ADD)
                nc.vector.tensor_scalar_add(O(1), i3[:], b_sb[:, co:co + 1])
                # out3 = (8v + u) + M5b
                stt(i3[:], i2[:], 8.0, i1[:], MULT, ADD)
                tt(O(3), i3[:], M(5), ADD)

                if chunk == 0:
                    nc.sync.dma_start(o_d[img, co, :, 0:1984], ob[:])
                else:
                    nc.sync.dma_start(o_d[img, co, :, 1984:3720],
                                      ob[:, 0:7])
                    nc.sync.dma_start(o_d[img, co, :, 3720:3844],
                                      ob[:, 7, 0:2])


def _build(reps: int = 1, hw_loop: bool = False, internal_io: bool = False):
    key = (reps, hw_loop, internal_io)
    if key in _CACHE:
        return _CACHE[key]

    nc = bacc.Bacc("TRN2", target_bir_lowering=False, debug=False,
                   num_devices=N_CORES)

    io_kind = "Internal" if internal_io else None
    x_d = nc.dram_tensor("xt", [IMG_PER_CORE, 2, 128, NXROW * 64], F16,
                         kind=io_kind or "ExternalInput")
    w_d = nc.dram_tensor("wt", [128, 2 * 4608], F16,
                         kind="ExternalInput")
    b_d = nc.dram_tensor("bias", [128, 2], F32, kind="ExternalInput")
    o_d = nc.dram_tensor("out", [IMG_PER_CORE, 2, 128, NV], F16,
                         kind=io_kind or "ExternalOutput")
    t_d = None
    if internal_io:
        t_d = nc.dram_tensor("tick", [128, 2], F32, kind="ExternalOutput")

    with tile.TileContext(nc) as tc:
        with tc.tile_pool(name="wp", bufs=1) as wp, \
             tc.tile_pool(name="xp", bufs=2) as xp, \
             tc.tile_pool(name="vp", bufs=2) as vp, \
             tc.tile_pool(name="tp", bufs=2) as tp, \
             tc.tile_pool(name="pp", bufs=8, space="PSUM") as pp, \
             tc.tile_pool(name="mp", bufs=4) as mp, \
             tc.tile_pool(name="op", bufs=4) as op:

            w_sb = wp.tile([128, 2 * 4608], F16, tag="w")
            nc.sync.dma_start(w_sb[:], w_d[:])
            b_sb = wp.tile([128, 2], F32, tag="bias")
            nc.sync.dma_start(b_sb[:], b_d[:])

            def rep():
                # w_sb free-dim layout: ct*4608 + p*768 + kw*256 + co*128
                # _emit_rep indexes w via a per-ct view
                _emit_rep(nc, x_d, o_d, w_sb, b_sb,
                          xp, vp, tp, pp, mp, op)

            if hw_loop and reps > 1:
                with tc.For_i(0, reps, 1):
                    rep()
            else:
                for _ in range(reps):
                    rep()

            if t_d is not None:
                nc.sync.dma_start(t_d[:], b_sb[:])

    nc.compile()
    _CACHE[key] = nc
    return nc


def _prep_inputs(x, weight, bias):
    """Host-side shard prep: threshold mask, G-transform of weights,
    transpose+pad of x. Per-core in_maps."""
    w = np.where(np.abs(weight) < SPARSE_TH, 0.0, weight).astype(np.float64)
    # wt[p, kw, cin, cout] = sum_kh G[p, kh] * w[cout, cin, kh, kw]
    wt = np.einsum('pr,ocrk->pkco', G_MAT, w)
    # layout [cin, ct? ...] -> [128, ct*4608 + p*768 + kw*256 + co*128 + o]
    # cin split: ct*128 + ci ; cout split: co*128 + o
    wt = wt.reshape(6, 3, 2, 128, 2, 128)          # p kw ct ci co o
    wt = wt.transpose(3, 2, 0, 1, 4, 5)            # ci ct p kw co o
    wt = np.ascontiguousarray(wt.reshape(128, 2 * 4608)).astype(np.float16)

    b2 = np.ascontiguousarray(
        bias.astype(np.float32).reshape(2, 128).T)

    n_img = x.shape[0]
    xs = x.astype(np.float32).reshape(n_img, 4096, C).transpose(0, 2, 1)
    xp = np.zeros((n_img, C, NXROW, 64), np.float16)
    xp[:, :, :64, :] = xs.reshape(n_img, C, 64, 64)
    xp = xp.reshape(n_img, 2, 128, NXROW * 64)

    in_maps = []
    for c in range(N_CORES):
        in_maps.append({
            "xt": np.ascontiguousarray(
                xp[c * IMG_PER_CORE:(c + 1) * IMG_PER_CORE]),
            "wt": wt,
            "bias": b2,
        })
    return in_maps


def _assemble(results):
    outs = np.concatenate([r["out"] for r in results], axis=0)
    outs = outs.astype(np.float32).reshape(32, C, 62, 62).transpose(0, 2, 3, 1)
    return np.ascontiguousarray(outs)


def kernel(x, weight, bias):
    x = np.asarray(x)
    weight = np.asarray(weight)
    bias = np.asarray(bias)
    nc = _build(reps=1)
    in_maps = _prep_inputs(x, weight, bias)
    res = run_bass_kernel_spmd(nc, in_maps, list(range(N_CORES)))
    return _assemble(res.results)


# revision 20
# speedup vs baseline: 1.4314x; 1.1167x over previous
"""Trainium2 Bass kernel: 3x3 VALID conv (NHWC, 256->256 ch) with weight
thresholding + bias, batch-sharded across 8 NeuronCores (4 images/core).

Algorithm: vertical 1D Winograd F(4,3) over kh — halves tensor-engine work
vs direct conv. All fp16 operands, fp32 PSUM accumulation (max rel err
~5e-3 vs the 2e-2 gate).

Engine split per core:
  - DVE: 8 fused scalar_tensor_tensor input-transform ops per (img, ct)
    + most of the inverse transform.
  - Pool (gpsimd): the plain tensor_tensor ops (4 of the input transform,
    u/v/t1/t2 of the inverse) — keeps DVE, the bottleneck, lighter.
  - PE: per (chunk of 8 vtiles, co, p): 6 accumulating fp16 matmuls
    (2 cin-tiles x 3 kw shifts) of 496 rows; FWL weight loads hidden.
  - ACT: PSUM -> fp16 M drains, plain Copy only (Identity+bias is 1.8x
    slower, so bias is folded into inverse stt ops instead).
"""

import sys

sys.path.insert(0, "/opt/trn_rl_repo")

import numpy as np

import concourse.bacc as bacc
import concourse.mybir as mybir
import concourse.tile as tile
from concourse.bass_utils import run_bass_kernel_spmd

F32 = mybir.dt.float32
F16 = mybir.dt.float16
ADD = mybir.AluOpType.add
SUB = mybir.AluOpType.subtract
MULT = mybir.AluOpType.mult
COPY = mybir.ActivationFunctionType.Copy
IDENT = mybir.ActivationFunctionType.Identity

N_CORES = 8
IMG_PER_CORE = 4
C = 256
NV = 62 * 62              # 3844 valid output pixels per image
NXROW = 66                # 64 input rows + 2 zero pad rows
SPARSE_TH = 0.01

G_MAT = np.array([
    [1 / 4, 0, 0],
    [-1 / 6, -1 / 6, -1 / 6],
    [-1 / 6, 1 / 6, -1 / 6],
    [1 / 24, 1 / 12, 1 / 6],
    [1 / 24, -1 / 12, 1 / 6],
    [0, 0, 1],
], dtype=np.float64)

_CACHE = {}

# Ablation knobs (timing experiments only; default = full kernel)
SKIP_TF = False
SKIP_INV = False
SKIP_MM = False


def _tf_ct(nc, x_d, xp, vp, tp, img, ct):
    """DMA x(img, ct) and emit its input transform. Returns the V tile."""
    stt = nc.vector.scalar_tensor_tensor
    ttp = nc.gpsimd.tensor_tensor

    xt = xp.tile([128, NXROW, 64], F16, tag=f"x{ct}", name="xt")
    nc.sync.dma_start(xt[:], x_d[img, ct])

    vt = vp.tile([128, 6, 16, 64], F16, tag=f"v{ct}", name="vt")
    t1 = tp.tile([128, 16, 64], F16, tag=f"t1{ct}", name="t1")
    t2 = tp.tile([128, 16, 64], F16, tag=f"t2{ct}", name="t2")

    def d(r):
        return xt[:, r:r + 61:4, :]

    V = lambda p: vt[:, p]
    if SKIP_TF:
        nc.vector.memset(vt[:], 0.0)
        return vt
    # V0 = 4 d0 - 5 d2 + d4 ; V5 = 4 d1 - 5 d3 + d5   (DVE)
    stt(t1[:], d(2), -5.0, d(4), MULT, ADD)
    stt(V(0), d(0), 4.0, t1[:], MULT, ADD)
    stt(t2[:], d(3), -5.0, d(5), MULT, ADD)
    stt(V(5), d(1), 4.0, t2[:], MULT, ADD)
    # s = d4 - 4 d2 ; p = 4 d1 - d3 (DVE) ; V1/V2 = s -/+ p (Pool)
    stt(t1[:], d(2), -4.0, d(4), MULT, ADD)
    stt(t2[:], d(1), 4.0, d(3), MULT, SUB)
    ttp(V(1), t1[:], t2[:], SUB)
    ttp(V(2), t1[:], t2[:], ADD)
    # u = d1 - d3 ; v = d4 - d2 (Pool) ; V3/V4 = -/+2u + v (DVE)
    ttp(t1[:], d(1), d(3), SUB)
    ttp(t2[:], d(4), d(2), SUB)
    stt(V(3), t1[:], -2.0, t2[:], MULT, ADD)
    stt(V(4), t1[:], 2.0, t2[:], MULT, ADD)
    return vt


def _tf_stage(nc, x_d, xp, vp, tp, img):
    return [_tf_ct(nc, x_d, xp, vp, tp, img, ct) for ct in range(2)]


def _emit_rep(nc, x_d, o_d, w_sb, b_sb, xp, vp, tp, pp, mp, op):
    stt = nc.vector.scalar_tensor_tensor
    ttv = nc.vector.tensor_tensor
    ttp = nc.gpsimd.tensor_tensor
    tsa = nc.vector.tensor_scalar_add

    v_cur = _tf_stage(nc, x_d, xp, vp, tp, 0)
    for img in range(IMG_PER_CORE):
        v_sb = v_cur
        v_cur = [None, None]

        # ---- matmuls + drains + inverse transform ----
        # Software pipelining: emit next image's transform (DVE/Pool)
        # before this image's inverse ops, so the vector engines run
        # ahead while the PE streams this image's matmuls.
        for chunk in range(2):
            vt0 = chunk * 8
            for co in range(2):
                if img + 1 < IMG_PER_CORE and chunk == 0:
                    # interleave next image's per-ct transform with this
                    # image's first two inverse groups
                    v_cur[co] = _tf_ct(nc, x_d, xp, vp, tp, img + 1, co)
                m = mp.tile([128, 6, 496], F16, tag="m", name="m")
                for p in range(6):
                    if SKIP_MM:
                        if p == 0:
                            nc.vector.memset(m[:], 0.0)
                        continue
                    ps = pp.tile([128, 496], F32, tag="ps", name="ps")
                    k = 0
                    for ct in range(2):
                        for kw in range(3):
                            off = ct * 4608 + p * 768 + kw * 256 + co * 128
                            nc.tensor.matmul(
                                ps[:],
                                w_sb[:, off:off + 128],
                                v_sb[ct][:, p, vt0:vt0 + 8, kw:kw + 62],
                                start=(k == 0), stop=(k == 5),
                            )
                            k += 1
                    nc.scalar.activation(m[:, p], ps[:], COPY)

                # inverse: out[4t+i] = AT[i] . M[:, t]  + bias
                ob = op.tile([128, 8, 4, 62], F16, tag="ob", name="ob")
                if SKIP_INV:
                    nc.vector.tensor_copy(ob[:], m[:, 0:4])
                    if chunk == 0:
                        nc.sync.dma_start(o_d[img, co, :, 0:1984], ob[:])
                    else:
                        nc.sync.dma_start(o_d[img, co, :, 1984:3720],
                                          ob[:, 0:7])
                        nc.sync.dma_start(o_d[img, co, :, 3720:3844],
                                          ob[:, 7, 0:2])
                    continue
                i1 = tp.tile([128, 496], F16, tag="i1", name="i1")
                i2 = tp.tile([128, 496], F16, tag="i2", name="i2")
                i3 = tp.tile([128, 496], F16, tag="i3", name="i3")
                M = lambda p: m[:, p]
                O = lambda i: ob[:, :, i, :]
                b = b_sb[:, co:co + 1]
                # t1 = M1+M2 ; t2 = M3+M4   (DVE)
                ttv(i1[:], M(1), M(2), ADD)
                ttv(i2[:], M(3), M(4), ADD)
                # out0 = ((M0 + b) + t1) + t2
                stt(i3[:], M(0), b, i1[:], ADD, ADD)
                ttv(O(0), i3[:], i2[:], ADD)
                # out2 = 4*t2 + t1 + b
                stt(i3[:], i2[:], 4.0, i1[:], MULT, ADD)
                tsa(O(2), i3[:], b)
                # u = M1-M2 ; v = M3-M4   (DVE)
                ttv(i1[:], M(1), M(2), SUB)
                ttv(i2[:], M(3), M(4), SUB)
                # out1 = 2v + u + b
                stt(i3[:], i2[:], 2.0, i1[:], MULT, ADD)
                tsa(O(1), i3[:], b)
                # out3 = (8v + u) + b + M5
                stt(i3[:], i2[:], 8.0, i1[:], MULT, ADD)
                stt(O(3), i3[:], b, M(5), ADD, ADD)

                if chunk == 0:
                    nc.sync.dma_start(o_d[img, co, :, 0:1984], ob[:])
                else:
                    nc.sync.dma_start(o_d[img, co, :, 1984:3720],
                                      ob[:, 0:7])
                    nc.sync.dma_start(o_d[img, co, :, 3720:3844],
                                      ob[:, 7, 0:2])


def _build(reps: int = 1, hw_loop: bool = False, internal_io: bool = False,
           unroll: int = 1):
    key = (reps, hw_loop, internal_io, unroll)
    if key in _CACHE:
        return _CACHE[key]

    nc = bacc.Bacc("TRN2", target_bir_lowering=False, debug=False,
                   num_devices=N_CORES)

    io_kind = "Internal" if internal_io else None
    x_d = nc.dram_tensor("xt", [IMG_PER_CORE, 2, 128, NXROW * 64], F16,
                         kind=io_kind or "ExternalInput")
    w_d = nc.dram_tensor("wt", [128, 2 * 4608], F16,
                         kind="ExternalInput")
    b_d = nc.dram_tensor("bias", [128, 2], F32, kind="ExternalInput")
    o_d = nc.dram_tensor("out", [IMG_PER_CORE, 2, 128, NV], F16,
                         kind=io_kind or "ExternalOutput")
    t_d = None
    if internal_io:
        t_d = nc.dram_tensor("tick", [128, 2], F32, kind="ExternalOutput")

    with tile.TileContext(nc) as tc:
        with tc.tile_pool(name="wp", bufs=1) as wp, \
             tc.tile_pool(name="xp", bufs=2) as xp, \
             tc.tile_pool(name="vp", bufs=2) as vp, \
             tc.tile_pool(name="tp", bufs=3) as tp, \
             tc.tile_pool(name="pp", bufs=8, space="PSUM") as pp, \
             tc.tile_pool(name="mp", bufs=6) as mp, \
             tc.tile_pool(name="op", bufs=6) as op:

            w_sb = wp.tile([128, 2 * 4608], F16, tag="w")
            nc.sync.dma_start(w_sb[:], w_d[:])
            b_sb = wp.tile([128, 2], F32, tag="bias")
            nc.sync.dma_start(b_sb[:], b_d[:])

            def rep():
                _emit_rep(nc, x_d, o_d, w_sb, b_sb,
                          xp, vp, tp, pp, mp, op)

            if hw_loop and reps > 1:
                assert reps % unroll == 0
                with tc.For_i(0, reps // unroll, 1):
                    for _ in range(unroll):
                        rep()
            else:
                for _ in range(reps):
                    rep()

            if t_d is not None:
                nc.sync.dma_start(t_d[:], b_sb[:])

    nc.compile()
    _CACHE[key] = nc
    return nc


def _prep_inputs(x, weight, bias):
    """Host-side shard prep: threshold mask, G-transform of weights,
    transpose+pad of x. Per-core in_maps."""
    w = np.where(np.abs(weight) < SPARSE_TH, 0.0, weight).astype(np.float64)
    wt = np.einsum('pr,ocrk->pkco', G_MAT, w)
    wt = wt.reshape(6, 3, 2, 128, 2, 128)          # p kw ct ci co o
    wt = wt.transpose(3, 2, 0, 1, 4, 5)            # ci ct p kw co o
    wt = np.ascontiguousarray(wt.reshape(128, 2 * 4608)).astype(np.float16)

    b2 = np.ascontiguousarray(
        bias.astype(np.float32).reshape(2, 128).T)

    n_img = x.shape[0]
    xs = x.astype(np.float32).reshape(n_img, 4096, C).transpose(0, 2, 1)
    xp = np.zeros((n_img, C, NXROW, 64), np.float16)
    xp[:, :, :64, :] = xs.reshape(n_img, C, 64, 64)
    xp = xp.reshape(n_img, 2, 128, NXROW * 64)

    in_maps = []
    for c in range(N_CORES):
        in_maps.append({
            "xt": np.ascontiguousarray(
                xp[c * IMG_PER_CORE:(c + 1) * IMG_PER_CORE]),
            "wt": wt,
            "bias": b2,
        })
    return in_maps


def _assemble(results):
    outs = np.concatenate([r["out"] for r in results], axis=0)
    outs = outs.astype(np.float32).reshape(32, C, 62, 62).transpose(0, 2, 3, 1)
    return np.ascontiguousarray(outs)


def kernel(x, weight, bias):
    x = np.asarray(x)
    weight = np.asarray(weight)
    bias = np.asarray(bias)
    nc = _build(reps=1)
    in_maps = _prep_inputs(x, weight, bias)
    res = run_bass_kernel_spmd(nc, in_maps, list(range(N_CORES)))
    return _assemble(res.results)
